# revision 5
# baseline (speedup 1.0000x reference)
"""AR(16) sampling kernel for 8 TRN2 NeuronCores.

Math: the reference runs a sequential scan
    y_t = sum_j a_j * y_{t-j} + eps_t        (a_j = coefficients[n-j])
The AR polynomial's roots (reciprocals of the generator's roots, which have
magnitude in [1.1, 2.0]) all lie inside |z| <= 0.91, so the impulse response
h decays below 1e-9 by lag 128 and below 1e-18 by lag 256.  The scan is
therefore exactly (to f32) a 256-tap causal FIR of the noise plus a
256-tap-decaying contribution of the initial state:

    y_t = sum_{d=0}^{255} h[d] * std * noise2[t-d]  +  sum_i G[i, t] * iv[b, i]

with noise2 = noise zero-padded by n=16 rows at the front.  On device, each
128x128 output tile (batch-major) is 2 accumulated matmuls over 128-row
noise chunks (plus one tiny G matmul for the first 256 columns) — fully
parallel over tiles, memory-bound by design.

Sharding: pure data parallelism, batch split 8 ways (512 rows/core).
"""

import os
import sys

import numpy as np

sys.path.insert(0, "/opt/trn_rl_repo")

N_CORES = 8
B_FULL = 4096
N_AR = 16
STEPS = 8192
B_SHARD = B_FULL // N_CORES  # 512

LAST_RESULTS = None  # BassKernelResults of the most recent run (for test.py)


def _build_nc(Bs: int, T: int):
    """Build the per-core Bass graph.  Bs = batch shard (mult of 128),
    T = padded time steps (mult of 512)."""
    import concourse.mybir as mybir
    from concourse import bacc
    from concourse.tile import TileContext

    f32 = mybir.dt.float32
    P = 128
    NB = Bs // P           # b-subtiles (4)
    TBLK = T // P          # 128-wide t blocks (64)
    WG = min(2048, T)      # t-group width for output stripes
    NTG = T // WG          # number of t-groups
    QW = 512               # psum unit width
    BLK_PER_Q = QW // P    # 4
    GROUP_ROWS = 512       # noise rows per DMA group
    CH_PER_G = GROUP_ROWS // P  # 4

    nc = bacc.Bacc()
    noise_d = nc.declare_dram_parameter("noise", [T, Bs], f32, isOutput=False)
    ivt_d = nc.declare_dram_parameter("ivt", [N_AR, Bs], f32, isOutput=False)
    hmat_d = nc.declare_dram_parameter("hmat", [P, 256], f32, isOutput=False)
    gmat_d = nc.declare_dram_parameter("gmat", [N_AR, 256], f32, isOutput=False)
    out_d = nc.declare_dram_parameter("out", [Bs, T], f32, isOutput=True)

    with TileContext(nc) as tc:
        with (
            tc.tile_pool(name="const", bufs=1) as cpool,
            tc.tile_pool(name="noise", bufs=6) as npool,
            tc.tile_pool(name="ostripe", bufs=8) as opool,
            tc.tile_pool(name="psum", bufs=4, space="PSUM") as ppool,
        ):
            hmat_t = cpool.tile([P, 256], f32)
            nc.sync.dma_start(out=hmat_t, in_=hmat_d[:, :])
            gmat_t = cpool.tile([N_AR, 256], f32)
            nc.sync.dma_start(out=gmat_t, in_=gmat_d[:, :])
            ivt_t = cpool.tile([N_AR, Bs], f32)
            nc.sync.dma_start(out=ivt_t, in_=ivt_d[:, :])

            noise_tiles = {}

            def load_group(g):
                t = npool.tile([P, CH_PER_G, Bs], f32, tag="noise")
                src = noise_d[GROUP_ROWS * g : GROUP_ROWS * (g + 1), :]
                nc.sync.dma_start(
                    out=t, in_=src.rearrange("(c p) b -> p c b", p=P)
                )
                noise_tiles[g] = t

            def chunk_ap(kc):
                # 128-row noise chunk kc as a [128, Bs] SBUF view
                g, r = divmod(kc, CH_PER_G)
                return noise_tiles[g][:, r, :]

            n_groups = T // GROUP_ROWS
            g_per_tg = WG // GROUP_ROWS

            for tg in range(NTG):
                for g in range(g_per_tg * tg, g_per_tg * (tg + 1)):
                    load_group(g)
                stripes = [
                    opool.tile([P, WG], f32, name=f"stripe{bs}", tag="stripe")
                    for bs in range(NB)
                ]
                for q in range(WG // QW):
                    for bs in range(NB):
                        ps = ppool.tile([P, QW], f32)
                        for j in range(BLK_PER_Q):
                            tb = tg * (WG // P) + q * BLK_PER_Q + j
                            out_sl = ps[:, j * P : (j + 1) * P]
                            mm = []
                            for c in (0, 1):
                                kc = tb - 1 + c
                                if kc < 0:
                                    continue
                                mm.append(
                                    (
                                        chunk_ap(kc)[:, bs * P : (bs + 1) * P],
                                        hmat_t[:, c * P : (c + 1) * P],
                                    )
                                )
                            if tb < 2:
                                mm.append(
                                    (
                                        ivt_t[:, bs * P : (bs + 1) * P],
                                        gmat_t[:, tb * P : (tb + 1) * P],
                                    )
                                )
                            for i, (lhsT, rhs) in enumerate(mm):
                                nc.tensor.matmul(
                                    out_sl,
                                    lhsT=lhsT,
                                    rhs=rhs,
                                    start=(i == 0),
                                    stop=(i == len(mm) - 1),
                                )
                        nc.vector.tensor_copy(
                            stripes[bs][:, q * QW : (q + 1) * QW], ps
                        )
                for bs in range(NB):
                    nc.sync.dma_start(
                        out=out_d[bs * P : (bs + 1) * P, tg * WG : (tg + 1) * WG],
                        in_=stripes[bs],
                    )
    nc.compile()
    return nc


def _host_matrices(coefficients: np.ndarray, log_noise_std: np.ndarray):
    """Impulse-response chunk matrices (f64 host math, cast to f32)."""
    n = N_AR
    co = coefficients.astype(np.float64)
    std = float(np.exp(log_noise_std.astype(np.float64))[0])
    L = 256
    h = np.zeros(L, np.float64)
    h[0] = 1.0
    for k in range(1, L):
        for j in range(1, min(k, n) + 1):
            h[k] += co[n - j] * h[k - j]
    hs = h * std
    # Hmat[k, c*128 + t] = h[t - k + 128 - 128*c] * std
    Hm = np.zeros((128, 256), np.float32)
    kk = np.arange(128)[:, None]
    tt = np.arange(128)[None, :]
    for c in (0, 1):
        d = tt - kk + 128 - 128 * c
        m = (d >= 0) & (d < L)
        blk = np.zeros((128, 128), np.float64)
        blk[m] = hs[d[m]]
        Hm[:, c * 128 : (c + 1) * 128] = blk.astype(np.float32)
    # G[i, t]: response at time t to unit initial value at slot i
    G = np.zeros((n, 256), np.float64)
    G[:, :n] = np.eye(n)
    for t in range(n, 256):
        G[:, t] = G[:, t - n : t] @ co
    return Hm, np.ascontiguousarray(G.astype(np.float32))


def kernel(initial_values, coefficients, log_noise_std, noise, steps):
    from concourse.bass_utils import run_bass_kernel_spmd

    global LAST_RESULTS

    initial_values = np.asarray(initial_values, dtype=np.float32)
    coefficients = np.asarray(coefficients, dtype=np.float32)
    log_noise_std = np.asarray(log_noise_std, dtype=np.float32)
    noise = np.asarray(noise, dtype=np.float32)

    Hm, Gm = _host_matrices(coefficients, log_noise_std)

    # zero-pad noise by n rows at the front -> (STEPS, B_FULL)
    noise2 = np.zeros((STEPS, B_FULL), np.float32)
    noise2[N_AR:] = noise
    ivT = np.ascontiguousarray(initial_values.T)  # (16, B_FULL)

    nc = _build_nc(B_SHARD, STEPS)
    in_maps = []
    for c in range(N_CORES):
        sl = slice(B_SHARD * c, B_SHARD * (c + 1))
        in_maps.append(
            {
                "noise": np.ascontiguousarray(noise2[:, sl]),
                "ivt": np.ascontiguousarray(ivT[:, sl]),
                "hmat": Hm,
                "gmat": Gm,
            }
        )

    trace = os.environ.get("KERNEL_TRACE", "0") == "1"
    res = run_bass_kernel_spmd(
        nc, in_maps, core_ids=list(range(N_CORES)), trace=trace
    )
    LAST_RESULTS = res

    out = np.empty((B_FULL, STEPS), np.float32)
    for c in range(N_CORES):
        out[B_SHARD * c : B_SHARD * (c + 1), :] = res.results[c]["out"]
    out[:, :N_AR] = initial_values
    return out


# revision 9
# speedup vs baseline: 1.3057x; 1.3057x over previous
"""AR(16) sampling kernel for 8 TRN2 NeuronCores.

Math: the reference runs a sequential scan
    y_t = sum_j a_j * y_{t-j} + eps_t        (a_j = coefficients[n-j])
The AR polynomial's roots (reciprocals of the generator's roots, which have
magnitude in [1.1, 2.0]) all lie inside |z| <= 0.91, so the impulse response
h decays below 1e-9 by lag 128 and below 1e-18 by lag 256.  The scan is
therefore exactly (to f32) a 256-tap causal FIR of the noise plus a
256-tap-decaying contribution of the initial state:

    y_t = sum_{d=0}^{255} h[d] * std * noise2[t-d]  +  sum_i G[i, t] * iv[b, i]

with noise2 = noise zero-padded by n=16 rows at the front.  On device, each
128x128 output tile (batch-major) is 2 accumulated matmuls over 128-row
noise chunks (plus one tiny G matmul for the first 256 columns) — fully
parallel over tiles, memory-bound by design.

Sharding: pure data parallelism, batch split 8 ways (512 rows/core).
"""

import os
import sys

import numpy as np

sys.path.insert(0, "/opt/trn_rl_repo")

N_CORES = 8
B_FULL = 4096
N_AR = 16
STEPS = 8192
B_SHARD = B_FULL // N_CORES  # 512

LAST_RESULTS = None  # BassKernelResults of the most recent run (for test.py)


def _build_nc(Bs: int, T: int, bf16_in: bool = False, bf16_out: bool = False):
    """Build the per-core Bass graph.  Bs = batch shard (mult of 128),
    T = padded time steps (mult of 512).  bf16_in: noise + band matrix
    stored/loaded as bf16 (matmul bf16, fp32 accumulate).  bf16_out:
    output stream stored as bf16 (host upcasts)."""
    import concourse.mybir as mybir
    from concourse import bacc
    from concourse.tile import TileContext

    f32 = mybir.dt.float32
    bf16 = mybir.dt.bfloat16
    in_dt = bf16 if bf16_in else f32
    out_dt = bf16 if bf16_out else f32
    P = 128
    NB = Bs // P           # b-subtiles (4)
    TBLK = T // P          # 128-wide t blocks (64)
    WG = min(2048, T)      # t-group width for output stripes
    NTG = T // WG          # number of t-groups
    QW = 512               # psum unit width
    BLK_PER_Q = QW // P    # 4
    GROUP_ROWS = 512       # noise rows per DMA group
    CH_PER_G = GROUP_ROWS // P  # 4

    nc = bacc.Bacc()
    noise_d = nc.declare_dram_parameter("noise", [T, Bs], in_dt, isOutput=False)
    ivt_d = nc.declare_dram_parameter("ivt", [N_AR, Bs], f32, isOutput=False)
    hmat_d = nc.declare_dram_parameter("hmat", [P, 256], in_dt, isOutput=False)
    gmat_d = nc.declare_dram_parameter("gmat", [N_AR, 256], f32, isOutput=False)
    out_d = nc.declare_dram_parameter("out", [Bs, T], out_dt, isOutput=True)

    with TileContext(nc) as tc:
        with (
            tc.tile_pool(name="const", bufs=1) as cpool,
            tc.tile_pool(name="noise", bufs=6) as npool,
            tc.tile_pool(name="ostripe", bufs=8) as opool,
            tc.tile_pool(name="psum", bufs=4, space="PSUM") as ppool,
        ):
            hmat_t = cpool.tile([P, 256], in_dt)
            nc.sync.dma_start(out=hmat_t, in_=hmat_d[:, :])
            gmat_t = cpool.tile([N_AR, 256], f32)
            nc.sync.dma_start(out=gmat_t, in_=gmat_d[:, :])
            ivt_t = cpool.tile([N_AR, Bs], f32)
            nc.sync.dma_start(out=ivt_t, in_=ivt_d[:, :])

            noise_tiles = {}

            def load_group(g):
                t = npool.tile([P, CH_PER_G, Bs], in_dt, tag="noise")
                src = noise_d[GROUP_ROWS * g : GROUP_ROWS * (g + 1), :]
                nc.sync.dma_start(
                    out=t, in_=src.rearrange("(c p) b -> p c b", p=P)
                )
                noise_tiles[g] = t

            def chunk_ap(kc):
                # 128-row noise chunk kc as a [128, Bs] SBUF view
                g, r = divmod(kc, CH_PER_G)
                return noise_tiles[g][:, r, :]

            n_groups = T // GROUP_ROWS
            g_per_tg = WG // GROUP_ROWS

            for tg in range(NTG):
                for g in range(g_per_tg * tg, g_per_tg * (tg + 1)):
                    load_group(g)
                stripes = [
                    opool.tile([P, WG], out_dt, name=f"stripe{bs}", tag="stripe")
                    for bs in range(NB)
                ]
                for q in range(WG // QW):
                    ti = tg * (WG // QW) + q   # 512-wide tile index
                    tb0 = ti * BLK_PER_Q
                    for bs in range(NB):
                        ps = ppool.tile([P, QW], f32)
                        bsl = slice(bs * P, (bs + 1) * P)
                        # 256-wide shingled band matmuls; first-touch
                        # stores (start=True marks whole bank pending),
                        # overlaps accumulate.
                        # evens first: they tile [0,512) exactly, so every
                        # later shingle hits a uniformly-written region
                        mm = []
                        for kc in (tb0, tb0 + 2, tb0 - 1, tb0 + 1, tb0 + 3):
                            if kc < 0:
                                continue
                            o = P * (kc - tb0)
                            if o == -P:
                                ps_sl, mv = ps[:, 0:P], hmat_t[:, P : 2 * P]
                            elif o == 3 * P:
                                ps_sl, mv = ps[:, 3 * P : 4 * P], hmat_t[:, 0:P]
                            else:
                                ps_sl, mv = ps[:, o : o + 2 * P], hmat_t[:, 0 : 2 * P]
                            mm.append((ps_sl, chunk_ap(kc)[:, bsl], mv))
                        if ti == 0:
                            mm.append(
                                (ps[:, 0 : 2 * P], ivt_t[:, bsl], gmat_t[:, 0 : 2 * P])
                            )
                        for i, (ps_sl, lhsT, rhs) in enumerate(mm):
                            nc.tensor.matmul(
                                ps_sl,
                                lhsT=lhsT,
                                rhs=rhs,
                                start=(i == 0),
                                stop=(i == len(mm) - 1),
                            )
                        nc.vector.tensor_copy(
                            stripes[bs][:, q * QW : (q + 1) * QW], ps
                        )
                for bs in range(NB):
                    nc.sync.dma_start(
                        out=out_d[bs * P : (bs + 1) * P, tg * WG : (tg + 1) * WG],
                        in_=stripes[bs],
                    )
    nc.compile()
    return nc


def _host_matrices(coefficients: np.ndarray, log_noise_std: np.ndarray):
    """Impulse-response chunk matrices (f64 host math, cast to f32)."""
    n = N_AR
    co = coefficients.astype(np.float64)
    std = float(np.exp(log_noise_std.astype(np.float64))[0])
    L = 256
    h = np.zeros(L, np.float64)
    h[0] = 1.0
    for k in range(1, L):
        for j in range(1, min(k, n) + 1):
            h[k] += co[n - j] * h[k - j]
    hs = h * std
    # band matrix: Hm[k, tau] = h[tau - k] * std  (256-wide shift-invariant)
    Hm = np.zeros((128, 256), np.float32)
    kk = np.arange(128)[:, None]
    tt = np.arange(256)[None, :]
    d = tt - kk
    m = (d >= 0) & (d < L)
    blk = np.zeros((128, 256), np.float64)
    blk[m] = hs[d[m]]
    Hm[:] = blk.astype(np.float32)
    # G[i, t]: response at time t to unit initial value at slot i
    G = np.zeros((n, 256), np.float64)
    G[:, :n] = np.eye(n)
    for t in range(n, 256):
        G[:, t] = G[:, t - n : t] @ co
    return Hm, np.ascontiguousarray(G.astype(np.float32))


BF16_IN = os.environ.get("KERNEL_BF16_IN", "0") == "1"
BF16_OUT = os.environ.get("KERNEL_BF16_OUT", "0") == "1"


def kernel(initial_values, coefficients, log_noise_std, noise, steps):
    import ml_dtypes

    from concourse.bass_utils import run_bass_kernel_spmd

    global LAST_RESULTS

    initial_values = np.asarray(initial_values, dtype=np.float32)
    coefficients = np.asarray(coefficients, dtype=np.float32)
    log_noise_std = np.asarray(log_noise_std, dtype=np.float32)
    noise = np.asarray(noise, dtype=np.float32)

    Hm, Gm = _host_matrices(coefficients, log_noise_std)

    # zero-pad noise by n rows at the front -> (STEPS, B_FULL)
    in_np_dt = ml_dtypes.bfloat16 if BF16_IN else np.float32
    noise2 = np.zeros((STEPS, B_FULL), in_np_dt)
    noise2[N_AR:] = noise.astype(in_np_dt)
    Hm = Hm.astype(in_np_dt)
    ivT = np.ascontiguousarray(initial_values.T)  # (16, B_FULL)

    nc = _build_nc(B_SHARD, STEPS, bf16_in=BF16_IN, bf16_out=BF16_OUT)
    in_maps = []
    for c in range(N_CORES):
        sl = slice(B_SHARD * c, B_SHARD * (c + 1))
        in_maps.append(
            {
                "noise": np.ascontiguousarray(noise2[:, sl]),
                "ivt": np.ascontiguousarray(ivT[:, sl]),
                "hmat": Hm,
                "gmat": Gm,
            }
        )

    trace = os.environ.get("KERNEL_TRACE", "0") == "1"
    res = run_bass_kernel_spmd(
        nc, in_maps, core_ids=list(range(N_CORES)), trace=trace
    )
    LAST_RESULTS = res

    out = np.empty((B_FULL, STEPS), np.float32)
    for c in range(N_CORES):
        out[B_SHARD * c : B_SHARD * (c + 1), :] = res.results[c]["out"].astype(
            np.float32
        )
    out[:, :N_AR] = initial_values
    return out
